# revision 16
# baseline (speedup 1.0000x reference)
"""Trainium2 Bass kernel for nn_CGATLayer (GNN message passing).

Algorithm (matches reference):
    z = feature @ fc_weight                      # [N, D]
    s = z @ attn[:D];  d = z @ attn[D:]          # per-node scalars
    e[n,j]   = leaky_relu(s[src[n,j]] + d[n])
    alpha[n,j] = sum_k relu(e[n,j] - e[n,k])
    h[n]     = sum_j alpha[n,j] * z[src[n,j]]

Device strategy (8 NeuronCores, SPMD single program):
  im2col formulation: instead of computing z per node and gathering rows
  per edge (DMA-descriptor bound: one descriptor per edge), the host ships
  the feature matrix with columns replicated in per-edge order
  (featdup[:, (t, j, p)] = feature[src[t*128+p, j]].T, j=32 = the dest's
  own feature column).  Each dest tile then needs 33 PE matmuls
  [128x128]x[128x66] against R = [fc | 0.5*fc@a1 | 0.5*fc@a2] to produce
  per-edge [z | s2 | d2] directly in PSUM -- per-edge data delivery rides
  on contiguous full-bandwidth DMA + the idle PE array instead of 200k
  512-byte gather descriptors.

  Using e' = e/2 (positive homogeneity of leaky_relu/relu):
      alpha = sum_k |e'_j - e'_k| + DEG*e'_j - sum_k e'_k
  so the pairwise clamp reduction is one abs-reduce.

  Per tile: 33 matmuls (5 PSUM banks) -> Act evacuates each bank
  transposed/bf16 into zsT [128, 66, 33] (d-major so the j-reduction is
  innermost) -> Act e' = Lrelu(s2 + d2) with free row-sum accumulator ->
  DVE pairwise diff (bf16 out) -> Pool abs-reduce -> DVE alpha, alpha
  broadcast to [128, 64, 32] (4x tensor_copy), product, one fold ->
  Pool reduces [128, 64, 16] -> h slot; h written to DRAM every 8 tiles
  in a partition-major layout so each partition's rows are contiguous.
"""

from contextlib import ExitStack

import numpy as np

import concourse.bass as bass
import concourse.bacc as bacc
import concourse.tile as tile
from concourse import mybir

F32 = mybir.dt.float32
BF16 = mybir.dt.bfloat16
ALU = mybir.AluOpType
AXL = mybir.AxisListType
ACT = mybir.ActivationFunctionType

N, DEG, IN_DIM, OUT_DIM = 50000, 32, 128, 64
NCORES = 8
NEG_SLOPE = 0.01
P = 128
PN = 6272                    # dest rows per core (49 tiles of 128)
NTILES = PN // P
JB = DEG + 1                 # 32 edge blocks + 1 own-feature block (d2)
OC = OUT_DIM + 2             # matmul output cols: z(64) | s2 | d2
HGRP = 8                     # tiles batched per h write


def build_program(pn=PN, in_dim=IN_DIM, out_dim=OUT_DIM, ncores=NCORES):
    ntiles = pn // P
    bank_js = [7, 7, 7, 7, 5]            # j-blocks per PSUM bank (sum=33)
    nh = (ntiles + HGRP - 1) // HGRP

    nc = bacc.Bacc("TRN2", num_devices=ncores)
    fd = nc.declare_dram_parameter("fd", [in_dim, ntiles * JB * P], BF16,
                                   isOutput=False)
    fc = nc.declare_dram_parameter("fc", [in_dim, out_dim], BF16, isOutput=False)
    fcT = nc.declare_dram_parameter("fcT", [out_dim, in_dim], BF16,
                                    isOutput=False)
    attn2 = nc.declare_dram_parameter("attn2", [out_dim, 2], BF16,
                                      isOutput=False)
    h = nc.declare_dram_parameter("h", [P, ntiles * out_dim], F32,
                                  isOutput=True)

    with tile.TileContext(nc) as tc, ExitStack() as ctx:
        const_pool = ctx.enter_context(tc.tile_pool(name="const", bufs=1))
        fd_pool = ctx.enter_context(tc.tile_pool(name="fd", bufs=3))
        ps_pool = ctx.enter_context(tc.tile_pool(name="ps", bufs=1,
                                                 space="PSUM"))
        zs_pool = ctx.enter_context(tc.tile_pool(name="zs", bufs=8))
        sm_pool = ctx.enter_context(tc.tile_pool(name="sm", bufs=7))
        D_pool = ctx.enter_context(tc.tile_pool(name="Dp", bufs=4))
        pr_pool = ctx.enter_context(tc.tile_pool(name="pr", bufs=4))
        h_pool = ctx.enter_context(tc.tile_pool(name="hp", bufs=2))

        # ---- weight prep: R = [fc | 0.5*fc@a1 | 0.5*fc@a2]  [in_dim, 66] bf16
        # (attn2 is pre-scaled by 0.5 on the host for both columns)
        fc_sb = const_pool.tile([in_dim, out_dim], BF16)
        nc.sync.dma_start(fc_sb[:], fc[:])
        fcT_sb = const_pool.tile([out_dim, in_dim], BF16)
        nc.sync.dma_start(fcT_sb[:], fcT[:])
        attn2_sb = const_pool.tile([out_dim, 2], BF16)
        nc.sync.dma_start(attn2_sb[:], attn2[:])
        R_sb = const_pool.tile([in_dim, OC], BF16)
        wsd_ps = ps_pool.tile([in_dim, 2], F32, tag="psW")
        nc.tensor.matmul(out=wsd_ps[:], lhsT=fcT_sb[:], rhs=attn2_sb[:],
                         start=True, stop=True)
        nc.vector.tensor_copy(out=R_sb[:, 0:out_dim], in_=fc_sb[:])
        nc.vector.tensor_copy(out=R_sb[:, out_dim:OC], in_=wsd_ps[:])

        st = {}

        def stage1(t):
            """DMA + matmuls + evac + e' for tile t."""
            fdt = fd_pool.tile([in_dim, JB * P], BF16, tag="fd")
            nc.sync.dma_start(fdt[:], fd[:, t * JB * P:(t + 1) * JB * P])

            # zsT[p, d, j]: d-major per-edge [z | s2 | d2] so the j-axis
            # (reduced later) is innermost
            zsT = zs_pool.tile([P, OC * JB], BF16, tag="zs")
            zsT3 = zsT[:].rearrange("p (d j) -> p d j", j=JB)
            j0 = 0
            for b, nj in enumerate(bank_js):
                ps = ps_pool.tile([P, nj * OC], F32, tag=f"ps{b}")
                for q in range(nj):
                    j = j0 + q
                    nc.tensor.matmul(out=ps[:, q * OC:(q + 1) * OC],
                                     lhsT=fdt[:, j * P:(j + 1) * P],
                                     rhs=R_sb[:], start=True, stop=True)
                # evacuate transposed (d-major) + f32 -> bf16
                ps3 = ps[:].rearrange("p (j d) -> p j d", d=OC)
                nc.scalar.activation(out=zsT3[:, :, j0:j0 + nj],
                                     in_=ps3.transpose([0, 2, 1]),
                                     func=ACT.Copy)
                j0 += nj

            # e' = Lrelu(s2 + d2); sumE = sum_j e'  (free accumulator)
            e = sm_pool.tile([P, DEG], F32, tag="e")
            sumE = sm_pool.tile([P, 1], F32, tag="sumE")
            s2v = zsT[:][:, out_dim * JB:out_dim * JB + DEG]
            d2v = zsT[:][:, (out_dim + 1) * JB + DEG:(out_dim + 1) * JB + DEG + 1]
            nc.scalar.activation(out=e[:], in_=s2v, func=ACT.Lrelu,
                                 bias=d2v, scale=1.0, alpha=NEG_SLOPE,
                                 accum_out=sumE[:])
            st[t] = {"zsT3": zsT3, "e": e, "sumE": sumE}

        def stage2(t):
            """D (Pool) + al0 (DVE), inputs one iteration old."""
            s = st[t]
            D = D_pool.tile([P, DEG * DEG], BF16, tag="D")
            D3 = D[:].rearrange("p (j k) -> p j k", k=DEG)
            e = s["e"]
            nc.gpsimd.tensor_tensor(
                out=D3, in0=e[:].unsqueeze(2).broadcast_to([P, DEG, DEG]),
                in1=e[:].unsqueeze(1).broadcast_to([P, DEG, DEG]),
                op=ALU.subtract)
            al0 = sm_pool.tile([P, DEG], F32, tag="al0")
            nc.vector.tensor_scalar(out=al0[:], in0=e[:], scalar1=float(DEG),
                                    scalar2=s["sumE"][:], op0=ALU.mult,
                                    op1=ALU.subtract)
            s["D3"], s["al0"] = D3, al0

        def stage3(t):
            """A = sum_k |D|  (DVE)."""
            s = st[t]
            A = sm_pool.tile([P, DEG], F32, tag="A")
            nc.vector.tensor_reduce(out=A[:], in_=s["D3"], axis=AXL.X,
                                    op=ALU.add, apply_absolute_value=True)
            s["A"] = A

        def stage4(t):
            """alpha = A + al0  (Pool; bf16 for the weighted sum)."""
            s = st[t]
            alpha = sm_pool.tile([P, DEG], BF16, tag="alpha")
            nc.gpsimd.tensor_tensor(out=alpha[:], in0=s["al0"][:],
                                    in1=s["A"][:], op=ALU.add)
            s["alpha"] = alpha

        def stage5(t):
            """prod + fold 32->16  (DVE); alpha rides as a broadcast view
            (middle-dim stride 0, packed last => 2x)."""
            s = st[t]
            prod = pr_pool.tile([P, out_dim * DEG], BF16, tag="prod")
            prod3 = prod[:].rearrange("p (d j) -> p d j", j=DEG)
            nc.vector.tensor_tensor(
                out=prod3, in0=s["zsT3"][:, 0:out_dim, 0:DEG],
                in1=s["alpha"][:].unsqueeze(1).broadcast_to(
                    [P, out_dim, DEG]),
                op=ALU.mult)
            ph = pr_pool.tile([P, out_dim * (DEG // 2)], BF16, tag="ph")
            ph3 = ph[:].rearrange("p (d j) -> p d j", j=DEG // 2)
            nc.vector.tensor_tensor(out=ph3, in0=prod3[:, :, 0:DEG // 2],
                                    in1=prod3[:, :, DEG // 2:DEG], op=ALU.add)
            s["ph3"] = ph3

        def stage6(t):
            """fold 16->8  (Pool)."""
            s = st[t]
            pq = pr_pool.tile([P, out_dim * (DEG // 4)], BF16, tag="pq")
            pq3 = pq[:].rearrange("p (d j) -> p d j", j=DEG // 4)
            nc.gpsimd.tensor_tensor(out=pq3, in0=s["ph3"][:, :, 0:DEG // 4],
                                    in1=s["ph3"][:, :, DEG // 4:DEG // 2],
                                    op=ALU.add)
            s["pq3"] = pq3

        def stage7(t):
            """reduce 8->1 into the h slot (DVE); h DMA every HGRP tiles."""
            nonlocal hbuf
            s = st.pop(t)
            if t % HGRP == 0:
                hbuf = h_pool.tile([P, HGRP * out_dim], F32, tag="hbuf")
            sl = t % HGRP
            nc.vector.tensor_reduce(
                out=hbuf[:, sl * out_dim:(sl + 1) * out_dim],
                in_=s["pq3"], axis=AXL.X, op=ALU.add)
            if t % HGRP == HGRP - 1 or t == ntiles - 1:
                g0 = (t // HGRP) * HGRP
                w = (t - g0 + 1) * out_dim
                nc.sync.dma_start(out=h[:, g0 * out_dim:g0 * out_dim + w],
                                  in_=hbuf[:, 0:w])

        hbuf = None
        phases = [(stage1, 0), (stage2, 1), (stage3, 3), (stage4, 4),
                  (stage5, 5), (stage6, 6), (stage7, 7)]
        depth = max(k for _, k in phases)
        for i in range(ntiles + depth):
            for phase, k in phases:
                t = i - k
                if 0 <= t < ntiles:
                    phase(t)

    nc.compile()
    return nc


def prep_inputs(feature, src_idx, fc_weight, attn_weight, ncores=NCORES):
    """Host-side sharding/layout prep -> per-core input maps."""
    import ml_dtypes

    bf16 = ml_dtypes.bfloat16
    feature = np.asarray(feature, dtype=np.float32)
    src = np.asarray(src_idx).astype(np.int64)
    fcw = np.asarray(fc_weight, dtype=np.float32)
    aw = np.asarray(attn_weight, dtype=np.float32)
    n, in_dim = feature.shape
    out_dim = fcw.shape[1]
    deg = src.shape[1]
    pn = PN
    ntiles = pn // P

    featT = np.ascontiguousarray(feature.T).astype(bf16)
    fcb = fcw.astype(bf16)
    fcT = np.ascontiguousarray(fcw.T).astype(bf16)
    attn2 = np.ascontiguousarray(
        np.stack([0.5 * aw[:out_dim, 0], 0.5 * aw[out_dim:, 0]],
                 axis=1)).astype(bf16)

    # padded per-core dest rows (cores own [c*pn, (c+1)*pn); rows >= n are
    # dummies whose outputs are discarded)
    src_pad = np.zeros((ncores * pn, deg), dtype=np.int64)
    src_pad[:n] = src
    featT_pad = np.zeros((in_dim, ncores * pn), dtype=bf16)
    featT_pad[:, :n] = featT

    in_maps = []
    for c in range(ncores):
        rows = np.arange(c * pn, (c + 1) * pn)
        cols = src_pad[c * pn:(c + 1) * pn]                 # [pn, deg]
        idx = np.concatenate([cols, rows[:, None]], axis=1)  # [pn, 33]
        # col order within a tile: j-major then p  -> (t, j, p)
        idx = idx.reshape(ntiles, P, JB).transpose(0, 2, 1).reshape(-1)
        fdc = np.ascontiguousarray(featT_pad[:, idx])
        in_maps.append({"fd": fdc, "fc": fcb, "fcT": fcT, "attn2": attn2})
    return in_maps, pn


_prog_cache = {}


def kernel(feature, src_idx, fc_weight, attn_weight):
    from concourse.bass_utils import run_bass_kernel_spmd

    in_maps, pn = prep_inputs(feature, src_idx, fc_weight, attn_weight)
    key = ("v6", feature.shape, pn)
    if key not in _prog_cache:
        _prog_cache[key] = build_program(pn=pn)
    nc = _prog_cache[key]
    res = run_bass_kernel_spmd(nc, in_maps, list(range(NCORES)))
    n = feature.shape[0]
    ntiles = pn // P
    h = np.zeros((NCORES * pn, OUT_DIM), dtype=np.float32)
    for c in range(NCORES):
        hc = np.asarray(res.results[c]["h"]).astype(np.float32)
        # h DRAM layout [128 p, ntiles*64] -> rows t*128+p
        hc = hc.reshape(P, ntiles, OUT_DIM).transpose(1, 0, 2).reshape(pn,
                                                                       OUT_DIM)
        h[c * pn:(c + 1) * pn] = hc
    return np.ascontiguousarray(h[:n], dtype=np.float32)
